# revision 1
# baseline (speedup 1.0000x reference)
"""Exp-domain CKY inside algorithm (ConstituencyTreeCRF log-partition) on TRN2.

Full input: scores [1024, 128, 128] f32. Output: logZ [1024] f32.
Data-parallel over batch: 8 NeuronCores x 128 batch elements each.

Math: alpha[k][s] = logsumexp_j(alpha[j][s] + alpha[k-1-j][s+j+1]) + score[s, s+k]
Worked in the exp domain with a per-width linear shift mu (keeps f32 in range;
for N(0,1) scores the shifted table stays within e^+-50):
  F_k[s] = exp(alpha_k[s] - mu*k);  F_0[s] = exp(score[s,s])
  F_k[s] = exp(score[s,s+k] - mu) * sum_j F_j[s] * F_{k-1-j}[s+j+1]
  logZ   = log(F_{n-1}[0]) + mu*(n-1)

Per-core layout: partition dim = batch (128); free dim holds tables:
  esc[b, s*n + t] = exp(scores[b,s,t] - mu)   (natural layout, exp'd in place)
  F  [b, k*n + s] = F_k[s]                    (diagonal-major)
Per width k (serial chain on VectorE):
  tmp[b, s, j] = F[j*n+s] * F[(k-1-j)*n + s+j+1]   one 3D-AP tensor_tensor
  W[b, s]      = sum_j tmp[b, s, j]                 tensor_reduce axis=X
  F[k*n+s]     = W * esc[s*(n+1) + k]               one tensor_tensor
"""

from contextlib import ExitStack

import numpy as np

import concourse.bacc as bacc
import concourse.bass as bass
import concourse.mybir as mybir
import concourse.tile as tile
from concourse import bass_utils

F32 = mybir.dt.float32
N = 128
B_FULL = 1024
N_CORES = 8
B_LOC = B_FULL // N_CORES
MU = 1.79


def build_nc(n: int = N, b: int = B_LOC, mu: float = MU):
    nc = bacc.Bacc("TRN2", target_bir_lowering=False, debug=False)
    sc_dram = nc.dram_tensor("scores", [b, n * n], F32, kind="ExternalInput")
    out_dram = nc.dram_tensor("logz", [b, 1], F32, kind="ExternalOutput")

    tmp_free = max(k * (n - k) for k in range(1, n))

    with tile.TileContext(nc) as tc, ExitStack() as ctx:
        pool = ctx.enter_context(tc.tile_pool(name="main", bufs=1))
        esc = pool.tile([b, n * n], F32)
        F = pool.tile([b, n * n], F32)
        tmp = pool.tile([b, tmp_free], F32)
        W = pool.tile([b, n], F32)
        lz = pool.tile([b, 1], F32)
        lz2 = pool.tile([b, 1], F32)
        neg_mu = pool.tile([b, 1], F32)
        nc.vector.memset(neg_mu[:], -float(mu))

        nc.sync.dma_start(esc[:], sc_dram[:])

        # F row 0 = exp(raw scores diagonal), before the in-place shift-exp
        diag_in = bass.AP(esc.tensor, 0, [[n * n, b], [n + 1, n]])
        nc.scalar.activation(F[:, 0:n], diag_in, mybir.ActivationFunctionType.Exp)
        # in-place esc = exp(scores - mu)
        nc.scalar.activation(
            esc[:], esc[:], mybir.ActivationFunctionType.Exp, bias=neg_mu[:]
        )

        for k in range(1, n):
            ns = n - k
            in0 = bass.AP(F.tensor, 0, [[n * n, b], [1, ns], [n, k]])
            in1 = bass.AP(
                F.tensor, (k - 1) * n + 1, [[n * n, b], [1, ns], [-(n - 1), k]]
            )
            t3 = bass.AP(tmp.tensor, 0, [[tmp_free, b], [k, ns], [1, k]])
            nc.vector.tensor_tensor(out=t3, in0=in0, in1=in1, op=mybir.AluOpType.mult)
            nc.vector.tensor_reduce(
                out=W[:, 0:ns], in_=t3, axis=mybir.AxisListType.X,
                op=mybir.AluOpType.add,
            )
            escd = bass.AP(esc.tensor, k, [[n * n, b], [n + 1, ns]])
            nc.vector.tensor_tensor(
                out=F[:, k * n : k * n + ns], in0=W[:, 0:ns], in1=escd,
                op=mybir.AluOpType.mult,
            )

        nc.scalar.activation(
            lz[:], F[:, (n - 1) * n : (n - 1) * n + 1],
            mybir.ActivationFunctionType.Ln,
        )
        nc.vector.tensor_scalar_add(lz2[:], lz[:], float(mu) * (n - 1))
        nc.sync.dma_start(out_dram[:], lz2[:])

    nc.compile()
    return nc


_NC_CACHE = {}


def _get_nc():
    if "nc" not in _NC_CACHE:
        _NC_CACHE["nc"] = build_nc()
    return _NC_CACHE["nc"]


def run_sharded(scores: np.ndarray, **spmd_kwargs):
    """Shard over batch, run on 8 cores, return (logZ [1024], BassKernelResults)."""
    nc = _get_nc()
    scores = np.ascontiguousarray(scores, dtype=np.float32)
    assert scores.shape == (B_FULL, N, N), scores.shape
    shards = scores.reshape(N_CORES, B_LOC, N * N)
    in_maps = [{"scores": np.ascontiguousarray(shards[c])} for c in range(N_CORES)]
    res = bass_utils.run_bass_kernel_spmd(
        nc, in_maps, core_ids=list(range(N_CORES)), **spmd_kwargs
    )
    out = np.concatenate(
        [np.asarray(r["logz"]).reshape(B_LOC) for r in res.results], axis=0
    )
    return out.astype(np.float32), res


def kernel(scores: np.ndarray) -> np.ndarray:
    out, _ = run_sharded(scores)
    return out
